# revision 15
# baseline (speedup 1.0000x reference)
"""Cayley soliton propagator on 8 Trainium2 NeuronCores.

Math: the Hamiltonian stencil H (jnp.roll-based) is a circulant matrix along D,
so the whole Cayley step (I + i*dt/2*H)^-1 (I - i*dt/2*H) is one complex
circulant matrix M, computed on the host from ham_w via an FFT of the stencil
symbol.  M's kernel decays exponentially; beyond +-H taps the dropped mass is
negligible vs fp16 noise, so applying M is a *banded* circulant matmul.

Device kernel, software-pipelined 5 deep so every cross-engine dependency has
at least one full chunk of slack (engines never head-block):
  stage s: intensity squares + ssum              (DVE/Pool/ACT)
  stage n: mean chain: PE ones-reduce -> denom -> recip -> broadcast (tiny ops)
  stage f: phs = ssum * minv (in-place)          (DVE)
  stage p: sin/cos (ACT) + rotation products     (DVE/Pool)
  stage q: banded matmul (PE, interleaved complex PSUM) + fp16 downcast copies
           (ACT) + fp16 DMA out (host upcasts to fp32)
Data-parallel over B*S rows across the 8 cores; psi is pre-transposed on the
host so the contraction axis D sits on SBUF partitions (no device transposes).
"""

import math

import numpy as np

import concourse.bass as bass
import concourse.bacc as bacc
import concourse.mybir as mybir
from concourse.bass_utils import run_bass_kernel_spmd
from concourse.tile import TileContext

B, S, D = 8, 2048, 1024
N_CORES = 8
ROWS = B * S // N_CORES          # rows (B*S systems) per core = 2048
RC = 256                         # row-chunk size (pipeline unit)
N_RC = ROWS // RC                # 8
N_DC = D // 128                  # 8 d-blocks of 128 partitions
NUM_SCALES, SPARSITY = 3, 5
HALF_DT = 0.05
F32 = mybir.dt.float32
F16 = mybir.dt.float16
AF = mybir.ActivationFunctionType
ALU = mybir.AluOpType

# tuning knobs: per-chunk engine placement (balance across ACT/DVE/Pool);
# early chunks avoid Pool (it starts late), tail chunks avoid Pool (drain)
SQR_ENGINE = {0: "act", 2: "act", 4: "act", 6: "act", 8: "act"}  # default: dve
SQI_ENGINE = {0: "dve", 8: "dve"}                                # default: pool
T4_ENGINE = {7: "dve", 8: "dve"}                                 # default: pool

_cache = {}


def _mm_pieces(dc, bw2):
    """Pieces of the interleaved band matmul for d-block dc.

    psum fp32 col = (256*dc + j2) mod 2048 for j2 in [0, bw2); split at the
    2048 wrap and the 512-fp32 PSUM bank boundary.  Returns (col, j2, width).
    """
    pieces = []
    j2 = 0
    while j2 < bw2:
        col = (256 * dc + j2) % 2048
        lim = min(bw2 - j2, 2048 - col, 512 - (col % 512))
        pieces.append((col, j2, lim))
        j2 += lim
    return pieces


def _build_program(uniform_alpha, H):
    BW = 128 + 2 * H
    BW2 = 2 * BW
    nc = bacc.Bacc()
    psi_rt = nc.dram_tensor("psi_rt", [D, ROWS], F16, kind="ExternalInput")
    psi_it = nc.dram_tensor("psi_it", [D, ROWS], F16, kind="ExternalInput")
    mband = nc.dram_tensor("mband", [128, 2 * BW2], F16, kind="ExternalInput")
    alpha_in = nc.dram_tensor("alpha", [D], F32, kind="ExternalInput")
    out = nc.dram_tensor("out", [ROWS, 2 * D], F16, kind="ExternalOutput")

    with TileContext(nc) as tc:
        with (
            tc.tile_pool(name="const", bufs=1) as constp,
            tc.tile_pool(name="ssump", bufs=4) as ssump,
            tc.tile_pool(name="work", bufs=2) as workp,
            tc.tile_pool(name="rot", bufs=2) as rotp,
            tc.tile_pool(name="xp", bufs=3) as xp,
            tc.tile_pool(name="small", bufs=4) as smallp,
            tc.tile_pool(name="outb", bufs=3) as outbp,
            tc.tile_pool(name="ps", bufs=7, space="PSUM") as psp,
            tc.tile_pool(name="psred", bufs=1, space="PSUM") as psredp,
        ):
            mband_sb = constp.tile([128, 2 * BW2], F16)
            alpha_sb = constp.tile([128, N_DC], F32)
            ones_col = constp.tile([128, 1], F16)
            nc.vector.memset(ones_col, 1.0)
            halfpi = constp.tile([128, 1], F32)
            nc.vector.memset(halfpi, math.pi / 2.0)
            zerob = constp.tile([128, 1], F32)
            nc.vector.memset(zerob, 0.0)

            # warm the ACT function table and the Pool ISA library off the
            # critical path (their lazy loads otherwise stall the first chunk)
            warm_a = constp.tile([1, 16], F16)
            warm_b = constp.tile([1, 16], F16)
            warm_c = constp.tile([128, 16], F16)
            nc.vector.memset(warm_a, 0.0)
            nc.scalar.activation(warm_b, warm_a, AF.Square)
            nc.scalar.activation(warm_b, warm_a, AF.Sin, bias=zerob[0:1, 0:1])
            nc.scalar.copy(warm_b, warm_a)
            nc.gpsimd.tensor_mul(warm_b, warm_a, warm_a)
            nc.gpsimd.partition_broadcast(warm_c[:, :], warm_a[:, :])

            # whole-tensor fp16 loads (host pre-casts), SBUF free = (dc, r);
            # first chunk's rows load before mband/alpha so compute starts early
            pr16 = constp.tile([128, N_DC * ROWS], F16)
            pi16 = constp.tile([128, N_DC * ROWS], F16)

            def load_rows(a, b):
                for dst, src in ((pi16, psi_it), (pr16, psi_rt)):
                    src_ap = src[:, :]
                    dst3 = dst.rearrange("p (dc r) -> p dc r", dc=N_DC)
                    nc.sync.dma_start(
                        out=dst3[:, :, a:b],
                        in_=bass.AP(
                            tensor=src_ap.tensor,
                            offset=src_ap.offset + a,
                            ap=[[ROWS, 128], [128 * ROWS, N_DC], [1, b - a]],
                        ),
                    )

            load_rows(0, RC)
            nc.sync.dma_start(
                out=alpha_sb, in_=alpha_in.rearrange("(dc p) -> p dc", p=128)
            )
            load_rows(RC, 2 * RC)
            nc.sync.dma_start(out=mband_sb, in_=mband[:, :])
            load_rows(2 * RC, 4 * RC)
            load_rows(4 * RC, ROWS)

            def chunk_view(tile, r0, rcw):
                ap = tile[:, :]
                return bass.AP(
                    tensor=ap.tensor,
                    offset=ap.offset + r0,
                    ap=[list(ap.ap[0]), [ROWS, N_DC], [1, rcw]],
                )

            chunks = [(i * RC, (i + 1) * RC) for i in range(N_RC - 1)]
            chunks += [(ROWS - RC, ROWS - RC // 2), (ROWS - RC // 2, ROWS)]
            NCH = len(chunks)

            st = [dict() for _ in range(NCH)]   # per-chunk live tiles

            def stage_s_sq(c):
                # squares; sq_r on DVE or ACT (balance), sq_i on Pool
                r0, r1 = chunks[c]
                RCW = r1 - r0
                W = N_DC * RCW
                prc = chunk_view(pr16, r0, RCW)
                pic = chunk_view(pi16, r0, RCW)
                sq_r = workp.tile([128, W], F16, tag="sq_r", name=f"sq_r_{c}")
                sq_i = workp.tile([128, W], F16, tag="sq_i", name=f"sq_i_{c}")
                eng = SQR_ENGINE.get(c, "dve")
                if eng == "act":
                    nc.scalar.activation(sq_r, prc, AF.Square)
                elif eng == "pool":
                    nc.gpsimd.tensor_mul(sq_r, prc, prc)
                else:
                    nc.vector.tensor_mul(sq_r, prc, prc)
                if SQI_ENGINE.get(c, "pool") == "dve":
                    nc.vector.tensor_mul(sq_i, pic, pic)
                else:
                    nc.gpsimd.tensor_mul(sq_i, pic, pic)
                st[c]["sq"] = (sq_r, sq_i)

            def stage_s_sum(c):
                r0, r1 = chunks[c]
                W = N_DC * (r1 - r0)
                sq_r, sq_i = st[c].pop("sq")
                ssum = ssump.tile([128, W], F16, tag="ssum", name=f"ssum_{c}")
                nc.vector.tensor_add(ssum, sq_r, sq_i)
                st[c]["ssum"] = ssum

            def stage_n_red(c):
                r0, r1 = chunks[c]
                RCW = r1 - r0
                ssum = st[c]["ssum"]
                ps_red = psredp.tile([1, RCW], F32, tag="psred", name=f"psred_{c}")
                for dc in range(N_DC):
                    nc.tensor.matmul(
                        ps_red,
                        ones_col,
                        ssum[:, dc * RCW : (dc + 1) * RCW],
                        start=(dc == 0),
                        stop=(dc == N_DC - 1),
                    )
                st[c]["psred"] = ps_red

            def stage_n_denom(c):
                r0, r1 = chunks[c]
                RCW = r1 - r0
                ps_red = st[c].pop("psred")
                denom = smallp.tile([1, RCW], F32, tag="denom", name=f"denom_{c}")
                nc.scalar.activation(
                    denom, ps_red, AF.Copy, bias=1e-8, scale=1.0 / float(D)
                )
                st[c]["denom"] = denom

            def stage_n_recip(c):
                r0, r1 = chunks[c]
                RCW = r1 - r0
                denom = st[c].pop("denom")
                rcp = smallp.tile([1, RCW], F32, tag="rcp", name=f"rcp_{c}")
                nc.vector.reciprocal(rcp, denom)
                minv16 = smallp.tile([1, RCW], F16, tag="minv16", name=f"minv16_{c}")
                with nc.allow_low_precision(reason="feeds fp16 phase"):
                    if uniform_alpha:
                        nc.vector.tensor_scalar(
                            minv16, rcp, alpha_sb[0:1, 0:1], None, op0=ALU.mult
                        )
                    else:
                        nc.vector.tensor_scalar(minv16, rcp, 1.0, None, op0=ALU.mult)
                st[c]["minv16"] = minv16

            def stage_n_bcast(c):
                r0, r1 = chunks[c]
                RCW = r1 - r0
                minv16 = st[c].pop("minv16")
                minv_bc = smallp.tile([128, RCW], F16, tag="minvbc", name=f"mbc_{c}")
                nc.gpsimd.partition_broadcast(minv_bc[:, :], minv16[:, :])
                st[c]["minv_bc"] = minv_bc

            def stage_f(c):
                # phs = ssum * minv, in place over ssum
                r0, r1 = chunks[c]
                RCW = r1 - r0
                ssum = st[c]["ssum"]
                minv_bc = st[c].pop("minv_bc")
                mb_ap = minv_bc[:, 0:RCW]
                minv_rep = bass.AP(
                    tensor=mb_ap.tensor,
                    offset=mb_ap.offset,
                    ap=[list(mb_ap.ap[0]), [0, N_DC], [1, RCW]],
                )
                nc.vector.tensor_mul(ssum, ssum, minv_rep)

            def stage_p_sins(c):
                r0, r1 = chunks[c]
                RCW = r1 - r0
                W = N_DC * RCW
                phs = st[c].pop("ssum")
                cc = rotp.tile([128, W], F16, tag="cc", name=f"cc_{c}")
                ss = rotp.tile([128, W], F16, tag="ss", name=f"ss_{c}")
                if uniform_alpha:
                    nc.scalar.activation(cc, phs, AF.Sin, bias=halfpi[:, 0:1])
                    nc.scalar.activation(ss, phs, AF.Sin, bias=zerob[:, 0:1])
                else:
                    for dc in range(N_DC):
                        sl = slice(dc * RCW, (dc + 1) * RCW)
                        nc.scalar.activation(
                            cc[:, sl], phs[:, sl], AF.Sin,
                            bias=halfpi[:, 0:1], scale=alpha_sb[:, dc : dc + 1],
                        )
                        nc.scalar.activation(
                            ss[:, sl], phs[:, sl], AF.Sin,
                            bias=zerob[:, 0:1], scale=alpha_sb[:, dc : dc + 1],
                        )
                st[c]["cs"] = (cc, ss)

            def stage_p_rot(c):
                # xr = pr*c - pi*s ; xi = pr*s + pi*c ; t4 on Pool, rest DVE
                r0, r1 = chunks[c]
                RCW = r1 - r0
                W = N_DC * RCW
                prc = chunk_view(pr16, r0, RCW)
                pic = chunk_view(pi16, r0, RCW)
                cc, ss = st[c].pop("cs")
                t1 = rotp.tile([128, W], F16, tag="t1", name=f"t1_{c}")
                t2 = rotp.tile([128, W], F16, tag="t2", name=f"t2_{c}")
                t3 = rotp.tile([128, W], F16, tag="t3", name=f"t3_{c}")
                t4 = rotp.tile([128, W], F16, tag="t4", name=f"t4_{c}")
                xr = xp.tile([128, W], F16, tag="xr", name=f"xr_{c}")
                xi = xp.tile([128, W], F16, tag="xi", name=f"xi_{c}")
                nc.vector.tensor_mul(t1, cc, prc)
                nc.vector.tensor_mul(t2, pic, ss)
                nc.vector.tensor_mul(t3, prc, ss)
                if T4_ENGINE.get(c, "pool") == "dve":
                    nc.vector.tensor_mul(t4, pic, cc)
                else:
                    nc.gpsimd.tensor_mul(t4, pic, cc)
                nc.vector.tensor_sub(xr, t1, t2)
                nc.vector.tensor_add(xi, t3, t4)
                st[c]["x"] = (xr, xi)

            def stage_q(c):
                # banded matmul into interleaved complex psum, in half-psum
                # units so copies overlap; then fp16 DMA out
                r0, r1 = chunks[c]
                RCW = r1 - r0
                xr, xi = st[c].pop("x")
                for rbl in range(RCW // 128):
                    quarters = []
                    for h in range(4):
                        pst = psp.tile(
                            [128, 512], F32, tag="ps", name=f"ps_{c}_{rbl}_{h}"
                        )
                        quarters.append(pst)
                    plan = {0: [], 1: [], 2: [], 3: []}
                    for dc in range(N_DC):
                        c0 = dc * RCW + rbl * 128
                        for xt, mat in ((xr, 0), (xi, 1)):
                            lhsT = xt[:, c0 : c0 + 128]
                            for col, j2, wdt in _mm_pieces(dc, BW2):
                                rhs = mband_sb[
                                    :, mat * BW2 + j2 : mat * BW2 + j2 + wdt
                                ]
                                plan[col // 512].append((col % 512, wdt, lhsT, rhs))
                    outbuf = outbp.tile([128, 2 * D], F16, tag="ob", name=f"ob_{c}_{rbl}")
                    for h in range(4):
                        plan[h].sort(key=lambda p: p[0])
                        for idx, (col, wdt, lhsT, rhs) in enumerate(plan[h]):
                            nc.tensor.matmul(
                                quarters[h][:, col : col + wdt],
                                lhsT,
                                rhs,
                                start=(idx == 0),
                                stop=(idx == len(plan[h]) - 1),
                                skip_group_check=True,
                            )
                        nc.scalar.copy(
                            outbuf[:, h * 512 : (h + 1) * 512], quarters[h][:, :]
                        )
                    # psum col s holds output fp16 col (s - 2H) mod 2048
                    rb = r0 // 128 + rbl
                    orow = out[rb * 128 : (rb + 1) * 128, :]
                    sh = 2 * H
                    nc.sync.dma_start(
                        out=orow[:, 0 : 2048 - sh], in_=outbuf[:, sh:2048]
                    )
                    nc.sync.dma_start(
                        out=orow[:, 2048 - sh : 2048], in_=outbuf[:, 0:sh]
                    )

            # 6-deep software pipeline: s(i) n(i-1) f(i-3) p(i-4) q(i-5);
            # the 2-iteration gaps n->f and f->p absorb per-iteration slip
            for i in range(NCH + 5):
                s, n, f, p, q = i, i - 1, i - 3, i - 4, i - 5
                if 0 <= p < NCH:
                    stage_p_sins(p)
                if 0 <= q < NCH:
                    stage_q(q)
                if 0 <= s < NCH:
                    stage_s_sq(s)
                if 0 <= p < NCH:
                    stage_p_rot(p)
                if 0 <= s < NCH:
                    stage_s_sum(s)
                if 0 <= f < NCH:
                    stage_f(f)
                if 0 <= n < NCH:
                    stage_n_red(n)
                    stage_n_denom(n)
                    stage_n_recip(n)
                    stage_n_bcast(n)
    return nc


def _host_ccol(ham_w):
    k = np.arange(D)
    lam = np.zeros(D, dtype=np.float64)
    w = np.asarray(ham_w, dtype=np.float64)
    for m in range(NUM_SCALES):
        for j in range(SPARSITY):
            off = (2 ** m) * (j + 1)
            lam += w[m, j] * 2.0 * (1.0 - np.cos(2.0 * np.pi * off * k / D))
    g = (1.0 - 1j * HALF_DT * lam) / (1.0 + 1j * HALF_DT * lam)
    return np.fft.ifft(g)


def _pick_H(ccol):
    mag = np.abs(ccol)
    half = np.minimum(np.arange(D), D - np.arange(D))
    for H in (24, 32, 48, 64, 96):
        if mag[half > H].sum() < 3e-4:
            return H
    return 128


def _host_mband(ccol, H):
    """Interleaved band tiles: R1 for xr (Mr, Mi pairs), R2 for xi (-Mi, Mr).

    R[p, 2j+c]: tap at k-d = j - H - p (shift-invariant across d-blocks);
    psum complex col q = (dc*128 + j) mod 1024 holds output k = (q - H) mod D.
    """
    BW = 128 + 2 * H
    rel = (np.arange(BW)[None, :] - H - np.arange(128)[:, None]) % D
    Mr = ccol.real[rel]
    Mi = ccol.imag[rel]
    R1 = np.empty((128, 2 * BW))
    R1[:, 0::2] = Mr
    R1[:, 1::2] = Mi
    R2 = np.empty((128, 2 * BW))
    R2[:, 0::2] = -Mi
    R2[:, 1::2] = Mr
    return np.concatenate([R1, R2], axis=1).astype(np.float16)


def kernel(psi_r, psi_i, alpha, ham_w):
    psi_r = np.asarray(psi_r, dtype=np.float32)
    psi_i = np.asarray(psi_i, dtype=np.float32)
    alpha = np.asarray(alpha, dtype=np.float32)

    ccol = _host_ccol(ham_w)
    H = _pick_H(ccol)
    uniform = bool(np.all(alpha == alpha.flat[0]))
    key = ("nc", uniform, H)
    if key not in _cache:
        nc = _build_program(uniform, H)
        nc.finalize()
        _cache[key] = nc
    nc = _cache[key]

    mband = _host_mband(ccol, H)
    prT = np.ascontiguousarray(psi_r.reshape(B * S, D).T.astype(np.float16))
    piT = np.ascontiguousarray(psi_i.reshape(B * S, D).T.astype(np.float16))

    in_maps = []
    for c in range(N_CORES):
        sl = slice(c * ROWS, (c + 1) * ROWS)
        in_maps.append(
            {
                "psi_rt": np.ascontiguousarray(prT[:, sl]),
                "psi_it": np.ascontiguousarray(piT[:, sl]),
                "mband": mband,
                "alpha": alpha,
            }
        )
    res = run_bass_kernel_spmd(nc, in_maps, core_ids=list(range(N_CORES)))
    _cache["last_run"] = res
    _cache["last_key"] = key
    full = np.concatenate([r["out"] for r in res.results], axis=0)
    return full.astype(np.float32).reshape(B, S, D, 2)


# revision 16
# speedup vs baseline: 1.0576x; 1.0576x over previous
"""Cayley soliton propagator on 8 Trainium2 NeuronCores.

Math: the Hamiltonian stencil H (jnp.roll-based) is a circulant matrix along D,
so the whole Cayley step (I + i*dt/2*H)^-1 (I - i*dt/2*H) is one complex
circulant matrix M, computed on the host from ham_w via an FFT of the stencil
symbol.  M's kernel decays exponentially; beyond +-H taps the dropped mass is
negligible vs fp16 noise, so applying M is a *banded* circulant matmul.

Device kernel, software-pipelined 5 deep so every cross-engine dependency has
at least one full chunk of slack (engines never head-block):
  stage s: intensity squares + ssum              (DVE/Pool/ACT)
  stage n: mean chain: PE ones-reduce -> denom -> recip -> broadcast (tiny ops)
  stage f: phs = ssum * minv (in-place)          (DVE)
  stage p: sin/cos (ACT) + rotation products     (DVE/Pool)
  stage q: banded matmul (PE, interleaved complex PSUM) + fp16 downcast copies
           (ACT) + fp16 DMA out (host upcasts to fp32)
Data-parallel over B*S rows across the 8 cores; psi is pre-transposed on the
host so the contraction axis D sits on SBUF partitions (no device transposes).
"""

import math

import numpy as np

import concourse.bass as bass
import concourse.bacc as bacc
import concourse.mybir as mybir
from concourse.bass_utils import run_bass_kernel_spmd
from concourse.tile import TileContext

B, S, D = 8, 2048, 1024
N_CORES = 8
ROWS = B * S // N_CORES          # rows (B*S systems) per core = 2048
RC = 256                         # row-chunk size (pipeline unit)
N_RC = ROWS // RC                # 8
N_DC = D // 128                  # 8 d-blocks of 128 partitions
NUM_SCALES, SPARSITY = 3, 5
HALF_DT = 0.05
F32 = mybir.dt.float32
F16 = mybir.dt.float16
AF = mybir.ActivationFunctionType
ALU = mybir.AluOpType

# tuning knobs: per-chunk engine placement (balance across ACT/DVE/Pool);
# early chunks avoid Pool (it starts late), tail chunks avoid Pool (drain)
SQR_ENGINE = {0: "act", 2: "act", 4: "act", 6: "act", 8: "act"}  # default: dve
SQI_ENGINE = {0: "dve", 8: "dve"}                                # default: pool
T4_ENGINE = {7: "dve", 8: "dve"}                                 # default: pool

_cache = {}


def _mm_pieces(dc, bw2):
    """Pieces of the interleaved band matmul for d-block dc.

    psum fp32 col = (256*dc + j2) mod 2048 for j2 in [0, bw2); split at the
    2048 wrap and the 512-fp32 PSUM bank boundary.  Returns (col, j2, width).
    """
    pieces = []
    j2 = 0
    while j2 < bw2:
        col = (256 * dc + j2) % 2048
        lim = min(bw2 - j2, 2048 - col, 512 - (col % 512))
        pieces.append((col, j2, lim))
        j2 += lim
    return pieces


def _build_program(uniform_alpha, H):
    BW = 128 + 2 * H
    BW2 = 2 * BW
    nc = bacc.Bacc()
    psi_rt = nc.dram_tensor("psi_rt", [D, ROWS], F16, kind="ExternalInput")
    psi_it = nc.dram_tensor("psi_it", [D, ROWS], F16, kind="ExternalInput")
    mband = nc.dram_tensor("mband", [128, 2 * BW2], F16, kind="ExternalInput")
    alpha_in = nc.dram_tensor("alpha", [D], F32, kind="ExternalInput")
    out = nc.dram_tensor("out", [ROWS, 2 * D], F16, kind="ExternalOutput")

    with TileContext(nc) as tc:
        with (
            tc.tile_pool(name="const", bufs=1) as constp,
            tc.tile_pool(name="ssump", bufs=4) as ssump,
            tc.tile_pool(name="work", bufs=2) as workp,
            tc.tile_pool(name="rot", bufs=2) as rotp,
            tc.tile_pool(name="xp", bufs=3) as xp,
            tc.tile_pool(name="small", bufs=4) as smallp,
            tc.tile_pool(name="outb", bufs=3) as outbp,
            tc.tile_pool(name="ps", bufs=3, space="PSUM") as psp,
            tc.tile_pool(name="psred", bufs=2, space="PSUM") as psredp,
        ):
            mband_sb = constp.tile([128, 2 * BW2], F16)
            alpha_sb = constp.tile([128, N_DC], F32)
            ones_col = constp.tile([128, 1], F16)
            nc.vector.memset(ones_col, 1.0)
            halfpi = constp.tile([128, 1], F32)
            nc.vector.memset(halfpi, math.pi / 2.0)
            zerob = constp.tile([128, 1], F32)
            nc.vector.memset(zerob, 0.0)

            # warm the ACT function table and the Pool ISA library off the
            # critical path (their lazy loads otherwise stall the first chunk)
            warm_a = constp.tile([1, 16], F16)
            warm_b = constp.tile([1, 16], F16)
            warm_c = constp.tile([128, 16], F16)
            nc.vector.memset(warm_a, 0.0)
            nc.scalar.activation(warm_b, warm_a, AF.Square)
            nc.scalar.activation(warm_b, warm_a, AF.Sin, bias=zerob[0:1, 0:1])
            nc.scalar.copy(warm_b, warm_a)
            nc.gpsimd.tensor_mul(warm_b, warm_a, warm_a)
            nc.gpsimd.partition_broadcast(warm_c[:, :], warm_a[:, :])

            # whole-tensor fp16 loads (host pre-casts), SBUF free = (dc, r);
            # first chunk's rows load before mband/alpha so compute starts early
            pr16 = constp.tile([128, N_DC * ROWS], F16)
            pi16 = constp.tile([128, N_DC * ROWS], F16)

            def load_rows(a, b):
                for dst, src in ((pi16, psi_it), (pr16, psi_rt)):
                    src_ap = src[:, :]
                    dst3 = dst.rearrange("p (dc r) -> p dc r", dc=N_DC)
                    nc.sync.dma_start(
                        out=dst3[:, :, a:b],
                        in_=bass.AP(
                            tensor=src_ap.tensor,
                            offset=src_ap.offset + a,
                            ap=[[ROWS, 128], [128 * ROWS, N_DC], [1, b - a]],
                        ),
                    )

            load_rows(0, RC)
            nc.sync.dma_start(
                out=alpha_sb, in_=alpha_in.rearrange("(dc p) -> p dc", p=128)
            )
            load_rows(RC, 2 * RC)
            nc.sync.dma_start(out=mband_sb, in_=mband[:, :])
            load_rows(2 * RC, 4 * RC)
            load_rows(4 * RC, ROWS)

            def chunk_view(tile, r0, rcw):
                ap = tile[:, :]
                return bass.AP(
                    tensor=ap.tensor,
                    offset=ap.offset + r0,
                    ap=[list(ap.ap[0]), [ROWS, N_DC], [1, rcw]],
                )

            chunks = [(i * RC, (i + 1) * RC) for i in range(N_RC - 1)]
            chunks += [(ROWS - RC, ROWS - RC // 2), (ROWS - RC // 2, ROWS)]
            NCH = len(chunks)

            st = [dict() for _ in range(NCH)]   # per-chunk live tiles

            def stage_s_sq(c):
                # squares; sq_r on DVE or ACT (balance), sq_i on Pool
                r0, r1 = chunks[c]
                RCW = r1 - r0
                W = N_DC * RCW
                prc = chunk_view(pr16, r0, RCW)
                pic = chunk_view(pi16, r0, RCW)
                sq_r = workp.tile([128, W], F16, tag="sq_r", name=f"sq_r_{c}")
                sq_i = workp.tile([128, W], F16, tag="sq_i", name=f"sq_i_{c}")
                eng = SQR_ENGINE.get(c, "dve")
                if eng == "act":
                    nc.scalar.activation(sq_r, prc, AF.Square)
                elif eng == "pool":
                    nc.gpsimd.tensor_mul(sq_r, prc, prc)
                else:
                    nc.vector.tensor_mul(sq_r, prc, prc)
                if SQI_ENGINE.get(c, "pool") == "dve":
                    nc.vector.tensor_mul(sq_i, pic, pic)
                else:
                    nc.gpsimd.tensor_mul(sq_i, pic, pic)
                st[c]["sq"] = (sq_r, sq_i)

            def stage_s_sum(c):
                r0, r1 = chunks[c]
                W = N_DC * (r1 - r0)
                sq_r, sq_i = st[c].pop("sq")
                ssum = ssump.tile([128, W], F16, tag="ssum", name=f"ssum_{c}")
                nc.vector.tensor_add(ssum, sq_r, sq_i)
                st[c]["ssum"] = ssum

            def stage_n_red(c):
                r0, r1 = chunks[c]
                RCW = r1 - r0
                ssum = st[c]["ssum"]
                ps_red = psredp.tile([1, RCW], F32, tag="psred", name=f"psred_{c}")
                for dc in range(N_DC):
                    nc.tensor.matmul(
                        ps_red,
                        ones_col,
                        ssum[:, dc * RCW : (dc + 1) * RCW],
                        start=(dc == 0),
                        stop=(dc == N_DC - 1),
                    )
                st[c]["psred"] = ps_red

            def stage_n_denom(c):
                r0, r1 = chunks[c]
                RCW = r1 - r0
                ps_red = st[c].pop("psred")
                denom = smallp.tile([1, RCW], F32, tag="denom", name=f"denom_{c}")
                nc.scalar.activation(
                    denom, ps_red, AF.Copy, bias=1e-8, scale=1.0 / float(D)
                )
                st[c]["denom"] = denom

            def stage_n_recip(c):
                r0, r1 = chunks[c]
                RCW = r1 - r0
                denom = st[c].pop("denom")
                rcp = smallp.tile([1, RCW], F32, tag="rcp", name=f"rcp_{c}")
                nc.vector.reciprocal(rcp, denom)
                minv16 = smallp.tile([1, RCW], F16, tag="minv16", name=f"minv16_{c}")
                with nc.allow_low_precision(reason="feeds fp16 phase"):
                    if uniform_alpha:
                        nc.vector.tensor_scalar(
                            minv16, rcp, alpha_sb[0:1, 0:1], None, op0=ALU.mult
                        )
                    else:
                        nc.vector.tensor_scalar(minv16, rcp, 1.0, None, op0=ALU.mult)
                st[c]["minv16"] = minv16

            def stage_n_bcast(c):
                r0, r1 = chunks[c]
                RCW = r1 - r0
                minv16 = st[c].pop("minv16")
                minv_bc = smallp.tile([128, RCW], F16, tag="minvbc", name=f"mbc_{c}")
                nc.gpsimd.partition_broadcast(minv_bc[:, :], minv16[:, :])
                st[c]["minv_bc"] = minv_bc

            def stage_f(c):
                # phs = ssum * minv, in place over ssum
                r0, r1 = chunks[c]
                RCW = r1 - r0
                ssum = st[c]["ssum"]
                minv_bc = st[c].pop("minv_bc")
                mb_ap = minv_bc[:, 0:RCW]
                minv_rep = bass.AP(
                    tensor=mb_ap.tensor,
                    offset=mb_ap.offset,
                    ap=[list(mb_ap.ap[0]), [0, N_DC], [1, RCW]],
                )
                nc.vector.tensor_mul(ssum, ssum, minv_rep)

            def stage_p_sins(c):
                r0, r1 = chunks[c]
                RCW = r1 - r0
                W = N_DC * RCW
                phs = st[c].pop("ssum")
                cc = rotp.tile([128, W], F16, tag="cc", name=f"cc_{c}")
                ss = rotp.tile([128, W], F16, tag="ss", name=f"ss_{c}")
                if uniform_alpha:
                    nc.scalar.activation(cc, phs, AF.Sin, bias=halfpi[:, 0:1])
                    nc.scalar.activation(ss, phs, AF.Sin, bias=zerob[:, 0:1])
                else:
                    for dc in range(N_DC):
                        sl = slice(dc * RCW, (dc + 1) * RCW)
                        nc.scalar.activation(
                            cc[:, sl], phs[:, sl], AF.Sin,
                            bias=halfpi[:, 0:1], scale=alpha_sb[:, dc : dc + 1],
                        )
                        nc.scalar.activation(
                            ss[:, sl], phs[:, sl], AF.Sin,
                            bias=zerob[:, 0:1], scale=alpha_sb[:, dc : dc + 1],
                        )
                st[c]["cs"] = (cc, ss)

            def stage_p_rot(c):
                # xr = pr*c - pi*s ; xi = pr*s + pi*c ; t4 on Pool, rest DVE
                r0, r1 = chunks[c]
                RCW = r1 - r0
                W = N_DC * RCW
                prc = chunk_view(pr16, r0, RCW)
                pic = chunk_view(pi16, r0, RCW)
                cc, ss = st[c].pop("cs")
                t1 = rotp.tile([128, W], F16, tag="t1", name=f"t1_{c}")
                t2 = rotp.tile([128, W], F16, tag="t2", name=f"t2_{c}")
                t3 = rotp.tile([128, W], F16, tag="t3", name=f"t3_{c}")
                t4 = rotp.tile([128, W], F16, tag="t4", name=f"t4_{c}")
                xr = xp.tile([128, W], F16, tag="xr", name=f"xr_{c}")
                xi = xp.tile([128, W], F16, tag="xi", name=f"xi_{c}")
                nc.vector.tensor_mul(t1, cc, prc)
                nc.vector.tensor_mul(t2, pic, ss)
                nc.vector.tensor_mul(t3, prc, ss)
                if T4_ENGINE.get(c, "pool") == "dve":
                    nc.vector.tensor_mul(t4, pic, cc)
                else:
                    nc.gpsimd.tensor_mul(t4, pic, cc)
                nc.vector.tensor_sub(xr, t1, t2)
                nc.vector.tensor_add(xi, t3, t4)
                st[c]["x"] = (xr, xi)

            def stage_q(c):
                # banded matmul into interleaved complex psum, in half-psum
                # units so copies overlap; then fp16 DMA out
                r0, r1 = chunks[c]
                RCW = r1 - r0
                xr, xi = st[c].pop("x")
                for rbl in range(RCW // 128):
                    halves = []
                    for h in range(2):
                        pst = psp.tile(
                            [128, 1024], F32, tag="ps", name=f"ps_{c}_{rbl}_{h}"
                        )
                        halves.append(pst)
                    plan = {0: [], 1: []}
                    for dc in range(N_DC):
                        c0 = dc * RCW + rbl * 128
                        for xt, mat in ((xr, 0), (xi, 1)):
                            lhsT = xt[:, c0 : c0 + 128]
                            for col, j2, wdt in _mm_pieces(dc, BW2):
                                rhs = mband_sb[
                                    :, mat * BW2 + j2 : mat * BW2 + j2 + wdt
                                ]
                                plan[col // 1024].append((col % 1024, wdt, lhsT, rhs))
                    outbuf = outbp.tile([128, 2 * D], F16, tag="ob", name=f"ob_{c}_{rbl}")
                    for h in range(2):
                        plan[h].sort(key=lambda p: p[0])
                        first, last = {}, {}
                        for idx, (col, wdt, *_r) in enumerate(plan[h]):
                            bank = col // 512
                            first.setdefault(bank, idx)
                            last[bank] = idx
                        for idx, (col, wdt, lhsT, rhs) in enumerate(plan[h]):
                            bank = col // 512
                            nc.tensor.matmul(
                                halves[h][:, col : col + wdt],
                                lhsT,
                                rhs,
                                start=(first[bank] == idx),
                                stop=(last[bank] == idx),
                                skip_group_check=True,
                            )
                        nc.scalar.copy(
                            outbuf[:, h * 1024 : (h + 1) * 1024], halves[h][:, :]
                        )
                    # psum col s holds output fp16 col (s - 2H) mod 2048
                    rb = r0 // 128 + rbl
                    orow = out[rb * 128 : (rb + 1) * 128, :]
                    sh = 2 * H
                    nc.sync.dma_start(
                        out=orow[:, 0 : 2048 - sh], in_=outbuf[:, sh:2048]
                    )
                    nc.sync.dma_start(
                        out=orow[:, 2048 - sh : 2048], in_=outbuf[:, 0:sh]
                    )

            # 6-deep software pipeline: s(i) n(i-1) f(i-3) p(i-4) q(i-5);
            # the 2-iteration gaps n->f and f->p absorb per-iteration slip
            for i in range(NCH + 5):
                s, n, f, p, q = i, i - 1, i - 3, i - 4, i - 5
                if 0 <= p < NCH:
                    stage_p_sins(p)
                if 0 <= q < NCH:
                    stage_q(q)
                if 0 <= s < NCH:
                    stage_s_sq(s)
                if 0 <= p < NCH:
                    stage_p_rot(p)
                if 0 <= s < NCH:
                    stage_s_sum(s)
                if 0 <= f < NCH:
                    stage_f(f)
                if 0 <= n < NCH:
                    stage_n_red(n)
                    stage_n_denom(n)
                    stage_n_recip(n)
                    stage_n_bcast(n)
    return nc


def _host_ccol(ham_w):
    k = np.arange(D)
    lam = np.zeros(D, dtype=np.float64)
    w = np.asarray(ham_w, dtype=np.float64)
    for m in range(NUM_SCALES):
        for j in range(SPARSITY):
            off = (2 ** m) * (j + 1)
            lam += w[m, j] * 2.0 * (1.0 - np.cos(2.0 * np.pi * off * k / D))
    g = (1.0 - 1j * HALF_DT * lam) / (1.0 + 1j * HALF_DT * lam)
    return np.fft.ifft(g)


def _pick_H(ccol):
    mag = np.abs(ccol)
    half = np.minimum(np.arange(D), D - np.arange(D))
    for H in (24, 32, 48, 64, 96):
        if mag[half > H].sum() < 3e-4:
            return H
    return 128


def _host_mband(ccol, H):
    """Interleaved band tiles: R1 for xr (Mr, Mi pairs), R2 for xi (-Mi, Mr).

    R[p, 2j+c]: tap at k-d = j - H - p (shift-invariant across d-blocks);
    psum complex col q = (dc*128 + j) mod 1024 holds output k = (q - H) mod D.
    """
    BW = 128 + 2 * H
    rel = (np.arange(BW)[None, :] - H - np.arange(128)[:, None]) % D
    Mr = ccol.real[rel]
    Mi = ccol.imag[rel]
    R1 = np.empty((128, 2 * BW))
    R1[:, 0::2] = Mr
    R1[:, 1::2] = Mi
    R2 = np.empty((128, 2 * BW))
    R2[:, 0::2] = -Mi
    R2[:, 1::2] = Mr
    return np.concatenate([R1, R2], axis=1).astype(np.float16)


def kernel(psi_r, psi_i, alpha, ham_w):
    psi_r = np.asarray(psi_r, dtype=np.float32)
    psi_i = np.asarray(psi_i, dtype=np.float32)
    alpha = np.asarray(alpha, dtype=np.float32)

    ccol = _host_ccol(ham_w)
    H = _pick_H(ccol)
    uniform = bool(np.all(alpha == alpha.flat[0]))
    key = ("nc", uniform, H)
    if key not in _cache:
        nc = _build_program(uniform, H)
        nc.finalize()
        _cache[key] = nc
    nc = _cache[key]

    mband = _host_mband(ccol, H)
    prT = np.ascontiguousarray(psi_r.reshape(B * S, D).T.astype(np.float16))
    piT = np.ascontiguousarray(psi_i.reshape(B * S, D).T.astype(np.float16))

    in_maps = []
    for c in range(N_CORES):
        sl = slice(c * ROWS, (c + 1) * ROWS)
        in_maps.append(
            {
                "psi_rt": np.ascontiguousarray(prT[:, sl]),
                "psi_it": np.ascontiguousarray(piT[:, sl]),
                "mband": mband,
                "alpha": alpha,
            }
        )
    res = run_bass_kernel_spmd(nc, in_maps, core_ids=list(range(N_CORES)))
    _cache["last_run"] = res
    _cache["last_key"] = key
    full = np.concatenate([r["out"] for r in res.results], axis=0)
    return full.astype(np.float32).reshape(B, S, D, 2)


# revision 18
# speedup vs baseline: 1.0794x; 1.0206x over previous
"""Cayley soliton propagator on 8 Trainium2 NeuronCores.

Math: the Hamiltonian stencil H (jnp.roll-based) is a circulant matrix along D,
so the whole Cayley step (I + i*dt/2*H)^-1 (I - i*dt/2*H) is one complex
circulant matrix M, computed on the host from ham_w via an FFT of the stencil
symbol.  M's kernel decays exponentially; beyond +-H taps the dropped mass is
negligible vs fp16 noise, so applying M is a *banded* circulant matmul.

Device kernel, software-pipelined 5 deep so every cross-engine dependency has
at least one full chunk of slack (engines never head-block):
  stage s: intensity squares + ssum              (DVE/Pool/ACT)
  stage n: mean chain: PE ones-reduce -> denom -> recip -> broadcast (tiny ops)
  stage f: phs = ssum * minv (in-place)          (DVE)
  stage p: sin/cos (ACT) + rotation products     (DVE/Pool)
  stage q: banded matmul (PE, interleaved complex PSUM) + fp16 downcast copies
           (ACT) + fp16 DMA out (host upcasts to fp32)
Data-parallel over B*S rows across the 8 cores; psi is pre-transposed on the
host so the contraction axis D sits on SBUF partitions (no device transposes).
"""

import math

import numpy as np

import concourse.bass as bass
import concourse.bacc as bacc
import concourse.mybir as mybir
from concourse.bass_utils import run_bass_kernel_spmd
from concourse.tile import TileContext

B, S, D = 8, 2048, 1024
N_CORES = 8
ROWS = B * S // N_CORES          # rows (B*S systems) per core = 2048
RC = 256                         # row-chunk size (pipeline unit)
N_RC = ROWS // RC                # 8
N_DC = D // 128                  # 8 d-blocks of 128 partitions
NUM_SCALES, SPARSITY = 3, 5
HALF_DT = 0.05
F32 = mybir.dt.float32
F16 = mybir.dt.float16
AF = mybir.ActivationFunctionType
ALU = mybir.AluOpType

# tuning knobs: per-chunk engine placement (balance across ACT/DVE/Pool);
# early chunks avoid Pool (it starts late), tail chunks avoid Pool (drain)
SQR_ENGINE = {0: "act", 2: "act", 4: "act", 6: "act", 8: "act"}  # default: dve
SQI_ENGINE = {0: "dve", 8: "dve"}                                # default: pool
T4_ENGINE = {7: "dve", 8: "dve"}                                 # default: pool
OFFSETS = (1, 3, 4, 6)          # pipeline stage offsets (n, f, p, q)

_cache = {}


def _mm_pieces(dc, bw2):
    """Pieces of the interleaved band matmul for d-block dc.

    psum fp32 col = (256*dc + j2) mod 2048 for j2 in [0, bw2); split at the
    2048 wrap and the 512-fp32 PSUM bank boundary.  Returns (col, j2, width).
    """
    pieces = []
    j2 = 0
    while j2 < bw2:
        col = (256 * dc + j2) % 2048
        lim = min(bw2 - j2, 2048 - col, 512 - (col % 512))
        pieces.append((col, j2, lim))
        j2 += lim
    return pieces


def _build_program(uniform_alpha, H):
    BW = 128 + 2 * H
    BW2 = 2 * BW
    nc = bacc.Bacc()
    psi_rt = nc.dram_tensor("psi_rt", [D, ROWS], F16, kind="ExternalInput")
    psi_it = nc.dram_tensor("psi_it", [D, ROWS], F16, kind="ExternalInput")
    mband = nc.dram_tensor("mband", [128, 2 * BW2], F16, kind="ExternalInput")
    alpha_in = nc.dram_tensor("alpha", [D], F32, kind="ExternalInput")
    out = nc.dram_tensor("out", [ROWS, 2 * D], F16, kind="ExternalOutput")

    with TileContext(nc) as tc:
        with (
            tc.tile_pool(name="const", bufs=1) as constp,
            tc.tile_pool(name="ssump", bufs=4) as ssump,
            tc.tile_pool(name="work", bufs=2) as workp,
            tc.tile_pool(name="rot", bufs=2) as rotp,
            tc.tile_pool(name="xp", bufs=3) as xp,
            tc.tile_pool(name="small", bufs=4) as smallp,
            tc.tile_pool(name="outb", bufs=3) as outbp,
            tc.tile_pool(name="ps", bufs=3, space="PSUM") as psp,
            tc.tile_pool(name="psred", bufs=2, space="PSUM") as psredp,
        ):
            mband_sb = constp.tile([128, 2 * BW2], F16)
            alpha_sb = constp.tile([128, N_DC], F32)
            ones_col = constp.tile([128, 1], F16)
            nc.vector.memset(ones_col, 1.0)
            halfpi = constp.tile([128, 1], F32)
            nc.vector.memset(halfpi, math.pi / 2.0)
            zerob = constp.tile([128, 1], F32)
            nc.vector.memset(zerob, 0.0)

            # warm the ACT function table and the Pool ISA library off the
            # critical path (their lazy loads otherwise stall the first chunk)
            warm_a = constp.tile([1, 16], F16)
            warm_b = constp.tile([1, 16], F16)
            warm_c = constp.tile([128, 16], F16)
            nc.vector.memset(warm_a, 0.0)
            nc.scalar.activation(warm_b, warm_a, AF.Square)
            nc.scalar.activation(warm_b, warm_a, AF.Sin, bias=zerob[0:1, 0:1])
            nc.scalar.copy(warm_b, warm_a)
            nc.gpsimd.tensor_mul(warm_b, warm_a, warm_a)
            nc.gpsimd.partition_broadcast(warm_c[:, :], warm_a[:, :])

            # whole-tensor fp16 loads (host pre-casts), SBUF free = (dc, r);
            # first chunk's rows load before mband/alpha so compute starts early
            pr16 = constp.tile([128, N_DC * ROWS], F16)
            pi16 = constp.tile([128, N_DC * ROWS], F16)

            def load_rows(a, b):
                for dst, src in ((pi16, psi_it), (pr16, psi_rt)):
                    src_ap = src[:, :]
                    dst3 = dst.rearrange("p (dc r) -> p dc r", dc=N_DC)
                    nc.sync.dma_start(
                        out=dst3[:, :, a:b],
                        in_=bass.AP(
                            tensor=src_ap.tensor,
                            offset=src_ap.offset + a,
                            ap=[[ROWS, 128], [128 * ROWS, N_DC], [1, b - a]],
                        ),
                    )

            load_rows(0, RC)
            nc.sync.dma_start(
                out=alpha_sb, in_=alpha_in.rearrange("(dc p) -> p dc", p=128)
            )
            load_rows(RC, 2 * RC)
            nc.sync.dma_start(out=mband_sb, in_=mband[:, :])
            load_rows(2 * RC, 4 * RC)
            load_rows(4 * RC, ROWS)

            def chunk_view(tile, r0, rcw):
                ap = tile[:, :]
                return bass.AP(
                    tensor=ap.tensor,
                    offset=ap.offset + r0,
                    ap=[list(ap.ap[0]), [ROWS, N_DC], [1, rcw]],
                )

            chunks = [(i * RC, (i + 1) * RC) for i in range(N_RC - 1)]
            chunks += [(ROWS - RC, ROWS - RC // 2), (ROWS - RC // 2, ROWS)]
            NCH = len(chunks)

            st = [dict() for _ in range(NCH)]   # per-chunk live tiles

            def stage_s_sq(c):
                # squares; sq_r on DVE or ACT (balance), sq_i on Pool
                r0, r1 = chunks[c]
                RCW = r1 - r0
                W = N_DC * RCW
                prc = chunk_view(pr16, r0, RCW)
                pic = chunk_view(pi16, r0, RCW)
                sq_r = workp.tile([128, W], F16, tag="sq_r", name=f"sq_r_{c}")
                sq_i = workp.tile([128, W], F16, tag="sq_i", name=f"sq_i_{c}")
                eng = SQR_ENGINE.get(c, "dve")
                if eng == "act":
                    nc.scalar.activation(sq_r, prc, AF.Square)
                elif eng == "pool":
                    nc.gpsimd.tensor_mul(sq_r, prc, prc)
                else:
                    nc.vector.tensor_mul(sq_r, prc, prc)
                if SQI_ENGINE.get(c, "pool") == "dve":
                    nc.vector.tensor_mul(sq_i, pic, pic)
                else:
                    nc.gpsimd.tensor_mul(sq_i, pic, pic)
                st[c]["sq"] = (sq_r, sq_i)

            def stage_s_sum(c):
                r0, r1 = chunks[c]
                W = N_DC * (r1 - r0)
                sq_r, sq_i = st[c].pop("sq")
                ssum = ssump.tile([128, W], F16, tag="ssum", name=f"ssum_{c}")
                nc.vector.tensor_add(ssum, sq_r, sq_i)
                st[c]["ssum"] = ssum

            def stage_n_red(c):
                r0, r1 = chunks[c]
                RCW = r1 - r0
                ssum = st[c]["ssum"]
                ps_red = psredp.tile([1, RCW], F32, tag="psred", name=f"psred_{c}")
                for dc in range(N_DC):
                    nc.tensor.matmul(
                        ps_red,
                        ones_col,
                        ssum[:, dc * RCW : (dc + 1) * RCW],
                        start=(dc == 0),
                        stop=(dc == N_DC - 1),
                    )
                st[c]["psred"] = ps_red

            def stage_n_denom(c):
                r0, r1 = chunks[c]
                RCW = r1 - r0
                ps_red = st[c].pop("psred")
                denom = smallp.tile([1, RCW], F32, tag="denom", name=f"denom_{c}")
                nc.scalar.activation(
                    denom, ps_red, AF.Copy, bias=1e-8, scale=1.0 / float(D)
                )
                st[c]["denom"] = denom

            def stage_n_recip(c):
                r0, r1 = chunks[c]
                RCW = r1 - r0
                denom = st[c].pop("denom")
                rcp = smallp.tile([1, RCW], F32, tag="rcp", name=f"rcp_{c}")
                nc.vector.reciprocal(rcp, denom)
                minv16 = smallp.tile([1, RCW], F16, tag="minv16", name=f"minv16_{c}")
                with nc.allow_low_precision(reason="feeds fp16 phase"):
                    if uniform_alpha:
                        nc.vector.tensor_scalar(
                            minv16, rcp, alpha_sb[0:1, 0:1], None, op0=ALU.mult
                        )
                    else:
                        nc.vector.tensor_scalar(minv16, rcp, 1.0, None, op0=ALU.mult)
                st[c]["minv16"] = minv16

            def stage_n_bcast(c):
                r0, r1 = chunks[c]
                RCW = r1 - r0
                minv16 = st[c].pop("minv16")
                minv_bc = smallp.tile([128, RCW], F16, tag="minvbc", name=f"mbc_{c}")
                nc.gpsimd.partition_broadcast(minv_bc[:, :], minv16[:, :])
                st[c]["minv_bc"] = minv_bc

            def stage_f(c):
                # phs = ssum * minv, in place over ssum
                r0, r1 = chunks[c]
                RCW = r1 - r0
                ssum = st[c]["ssum"]
                minv_bc = st[c].pop("minv_bc")
                mb_ap = minv_bc[:, 0:RCW]
                minv_rep = bass.AP(
                    tensor=mb_ap.tensor,
                    offset=mb_ap.offset,
                    ap=[list(mb_ap.ap[0]), [0, N_DC], [1, RCW]],
                )
                nc.vector.tensor_mul(ssum, ssum, minv_rep)

            def stage_p_sins(c):
                r0, r1 = chunks[c]
                RCW = r1 - r0
                W = N_DC * RCW
                phs = st[c].pop("ssum")
                cc = rotp.tile([128, W], F16, tag="cc", name=f"cc_{c}")
                ss = rotp.tile([128, W], F16, tag="ss", name=f"ss_{c}")
                if uniform_alpha:
                    nc.scalar.activation(cc, phs, AF.Sin, bias=halfpi[:, 0:1])
                    nc.scalar.activation(ss, phs, AF.Sin, bias=zerob[:, 0:1])
                else:
                    for dc in range(N_DC):
                        sl = slice(dc * RCW, (dc + 1) * RCW)
                        nc.scalar.activation(
                            cc[:, sl], phs[:, sl], AF.Sin,
                            bias=halfpi[:, 0:1], scale=alpha_sb[:, dc : dc + 1],
                        )
                        nc.scalar.activation(
                            ss[:, sl], phs[:, sl], AF.Sin,
                            bias=zerob[:, 0:1], scale=alpha_sb[:, dc : dc + 1],
                        )
                st[c]["cs"] = (cc, ss)

            def stage_p_rot(c):
                # xr = pr*c - pi*s ; xi = pr*s + pi*c ; t4 on Pool, rest DVE
                r0, r1 = chunks[c]
                RCW = r1 - r0
                W = N_DC * RCW
                prc = chunk_view(pr16, r0, RCW)
                pic = chunk_view(pi16, r0, RCW)
                cc, ss = st[c].pop("cs")
                t1 = rotp.tile([128, W], F16, tag="t1", name=f"t1_{c}")
                t2 = rotp.tile([128, W], F16, tag="t2", name=f"t2_{c}")
                t3 = rotp.tile([128, W], F16, tag="t3", name=f"t3_{c}")
                t4 = rotp.tile([128, W], F16, tag="t4", name=f"t4_{c}")
                xr = xp.tile([128, W], F16, tag="xr", name=f"xr_{c}")
                xi = xp.tile([128, W], F16, tag="xi", name=f"xi_{c}")
                nc.vector.tensor_mul(t1, cc, prc)
                nc.vector.tensor_mul(t2, pic, ss)
                nc.vector.tensor_mul(t3, prc, ss)
                if T4_ENGINE.get(c, "pool") == "dve":
                    nc.vector.tensor_mul(t4, pic, cc)
                else:
                    nc.gpsimd.tensor_mul(t4, pic, cc)
                nc.vector.tensor_sub(xr, t1, t2)
                nc.vector.tensor_add(xi, t3, t4)
                st[c]["x"] = (xr, xi)

            def stage_q(c):
                # banded matmul into interleaved complex psum, in half-psum
                # units so copies overlap; then fp16 DMA out
                r0, r1 = chunks[c]
                RCW = r1 - r0
                xr, xi = st[c].pop("x")
                for rbl in range(RCW // 128):
                    halves = []
                    for h in range(2):
                        pst = psp.tile(
                            [128, 1024], F32, tag="ps", name=f"ps_{c}_{rbl}_{h}"
                        )
                        halves.append(pst)
                    plan = {0: [], 1: []}
                    for dc in range(N_DC):
                        c0 = dc * RCW + rbl * 128
                        for xt, mat in ((xr, 0), (xi, 1)):
                            lhsT = xt[:, c0 : c0 + 128]
                            for col, j2, wdt in _mm_pieces(dc, BW2):
                                rhs = mband_sb[
                                    :, mat * BW2 + j2 : mat * BW2 + j2 + wdt
                                ]
                                plan[col // 1024].append((col % 1024, wdt, lhsT, rhs))
                    outbuf = outbp.tile([128, 2 * D], F16, tag="ob", name=f"ob_{c}_{rbl}")
                    for h in range(2):
                        plan[h].sort(key=lambda p: p[0])
                        first, last = {}, {}
                        for idx, (col, wdt, *_r) in enumerate(plan[h]):
                            bank = col // 512
                            first.setdefault(bank, idx)
                            last[bank] = idx
                        for idx, (col, wdt, lhsT, rhs) in enumerate(plan[h]):
                            bank = col // 512
                            nc.tensor.matmul(
                                halves[h][:, col : col + wdt],
                                lhsT,
                                rhs,
                                start=(first[bank] == idx),
                                stop=(last[bank] == idx),
                                skip_group_check=True,
                            )
                        nc.scalar.copy(
                            outbuf[:, h * 1024 : (h + 1) * 1024], halves[h][:, :]
                        )
                    # psum col s holds output fp16 col (s - 2H) mod 2048
                    rb = r0 // 128 + rbl
                    orow = out[rb * 128 : (rb + 1) * 128, :]
                    sh = 2 * H
                    nc.sync.dma_start(
                        out=orow[:, 0 : 2048 - sh], in_=outbuf[:, sh:2048]
                    )
                    nc.sync.dma_start(
                        out=orow[:, 2048 - sh : 2048], in_=outbuf[:, 0:sh]
                    )

            # 6-deep software pipeline: s(i) n(i-1) f(i-3) p(i-4) q(i-5);
            # the 2-iteration gaps n->f and f->p absorb per-iteration slip
            on, of, op_, oq = OFFSETS
            for i in range(NCH + oq):
                s, n, f, p, q = i, i - on, i - of, i - op_, i - oq
                if 0 <= p < NCH:
                    stage_p_sins(p)
                if 0 <= q < NCH:
                    stage_q(q)
                if 0 <= s < NCH:
                    stage_s_sq(s)
                if 0 <= p < NCH:
                    stage_p_rot(p)
                if 0 <= s < NCH:
                    stage_s_sum(s)
                if 0 <= f < NCH:
                    stage_f(f)
                if 0 <= n < NCH:
                    stage_n_red(n)
                    stage_n_denom(n)
                    stage_n_recip(n)
                    stage_n_bcast(n)
    return nc


def _host_ccol(ham_w):
    k = np.arange(D)
    lam = np.zeros(D, dtype=np.float64)
    w = np.asarray(ham_w, dtype=np.float64)
    for m in range(NUM_SCALES):
        for j in range(SPARSITY):
            off = (2 ** m) * (j + 1)
            lam += w[m, j] * 2.0 * (1.0 - np.cos(2.0 * np.pi * off * k / D))
    g = (1.0 - 1j * HALF_DT * lam) / (1.0 + 1j * HALF_DT * lam)
    return np.fft.ifft(g)


def _pick_H(ccol):
    mag = np.abs(ccol)
    half = np.minimum(np.arange(D), D - np.arange(D))
    for H in (24, 32, 48, 64, 96):
        if mag[half > H].sum() < 3e-4:
            return H
    return 128


def _host_mband(ccol, H):
    """Interleaved band tiles: R1 for xr (Mr, Mi pairs), R2 for xi (-Mi, Mr).

    R[p, 2j+c]: tap at k-d = j - H - p (shift-invariant across d-blocks);
    psum complex col q = (dc*128 + j) mod 1024 holds output k = (q - H) mod D.
    """
    BW = 128 + 2 * H
    rel = (np.arange(BW)[None, :] - H - np.arange(128)[:, None]) % D
    Mr = ccol.real[rel]
    Mi = ccol.imag[rel]
    R1 = np.empty((128, 2 * BW))
    R1[:, 0::2] = Mr
    R1[:, 1::2] = Mi
    R2 = np.empty((128, 2 * BW))
    R2[:, 0::2] = -Mi
    R2[:, 1::2] = Mr
    return np.concatenate([R1, R2], axis=1).astype(np.float16)


def kernel(psi_r, psi_i, alpha, ham_w):
    psi_r = np.asarray(psi_r, dtype=np.float32)
    psi_i = np.asarray(psi_i, dtype=np.float32)
    alpha = np.asarray(alpha, dtype=np.float32)

    ccol = _host_ccol(ham_w)
    H = _pick_H(ccol)
    uniform = bool(np.all(alpha == alpha.flat[0]))
    key = ("nc", uniform, H)
    if key not in _cache:
        nc = _build_program(uniform, H)
        nc.finalize()
        _cache[key] = nc
    nc = _cache[key]

    mband = _host_mband(ccol, H)
    prT = np.ascontiguousarray(psi_r.reshape(B * S, D).T.astype(np.float16))
    piT = np.ascontiguousarray(psi_i.reshape(B * S, D).T.astype(np.float16))

    in_maps = []
    for c in range(N_CORES):
        sl = slice(c * ROWS, (c + 1) * ROWS)
        in_maps.append(
            {
                "psi_rt": np.ascontiguousarray(prT[:, sl]),
                "psi_it": np.ascontiguousarray(piT[:, sl]),
                "mband": mband,
                "alpha": alpha,
            }
        )
    res = run_bass_kernel_spmd(nc, in_maps, core_ids=list(range(N_CORES)))
    _cache["last_run"] = res
    _cache["last_key"] = key
    full = np.concatenate([r["out"] for r in res.results], axis=0)
    return full.astype(np.float32).reshape(B, S, D, 2)
